# revision 16
# baseline (speedup 1.0000x reference)
"""KAN group-spline kernel for Trainium2 (8 NeuronCores, data-parallel over batch).

Math: out = id_gain[c]*x + F_c(v) + bias[c], v = 15.5*a*x + oc_c, F_c = channel's
cubic spline (32-knot uniform B-spline, constant outside v in [0,33]).

v2 — wide-segment quartic telescoping (8 segments instead of 33):

  F_c(v) = A_c + sum_p gamma_{c,p} * q_{c,p}(r_p),   r_p = clamp((v-S_p)/w_p, 0, 1)
  q(m)   = m^4 + a3 m^3 + a2 m^2 + a1 m    (normalized: leading coeff == 1)

The quartic over w_p=4..5 knots is a host-side weighted LSQ fit of the exact
spline increment (rel err ~7e-3 vs 2e-2 budget); saturation is exact via
telescoping + host-side drift correction of the device's bf16-rounded
endpoint values.

Engine plan per 128xNCOL tile (segments independent -> full pipelining):
  ACT : h_p = Relu((15.5/w_p)*x + (oc_c - S_p)/w_p)   [per-partition bias AP]
  DVE : y_p = q(min(h_p, 1))      [custom 7-stage TTSS op "KAN_Q4";
        s0=a1, s1=a2, in1=a3 spill APs; bf16 out]
  PE  : psum  = diag(ig) @ x_chunk   [fp32, start]
        psum += diag(bias+A) @ ones  [fp32]
        psum += diag(gamma_p) @ y_p  [bf16, 8x]
  ACT : out_sbuf = Copy(psum)        [evacuate]

8 DVE passes + 9 ACT passes + 10 PE matmul-chunks per tile, vs the v1
33/2/35 split — DVE was the 1x-mode bottleneck (8-stage custom ops can't
use 2x/4x perf modes). Measured 1144136 ns (vs 3806968 ns baseline), rel
err 6.5e-3. NOTE: a 7-segment layout ([5,5,4,4,5,5,5], rel 1.0e-2) measured
SLOWER (1.38ms) despite fewer passes — do not "optimize" the pass count
down without re-measuring; folding bias2 into an Identity-evac with AP
bias also regressed (1.50ms).
"""

import os
import numpy as np

B, C, H, W = 16, 192, 128, 128
K, G = 32, 32
NSEG = 33
NCORES = 8
ROWS = (B // NCORES) * C           # 384 rows per core
FREE = H * W                       # 16384
NCOL = int(os.environ.get("KAN_NCOL", "2048"))
COLT = FREE // NCOL
ROWT = ROWS // 128                 # 3
MMF = 512                          # matmul free chunk / PSUM bank (fp32 out)
NMM = NCOL // MMF

WIDTHS = tuple(int(w) for w in os.environ.get("KAN_WIDTHS", "4,4,4,4,4,4,4,5").split(","))
assert sum(WIDTHS) == NSEG
NSEGP = len(WIDTHS)
STARTS = tuple(int(s) for s in np.concatenate([[0], np.cumsum(WIDTHS)])[:-1])

# tab layout per pair p: [actbias, a1, a2, a3]
NTAB = 4 * NSEGP
NW32 = 2 * ROWT                    # per-rowtile {ig, bias2} diags (fp32)
NWBF = ROWT * NSEGP                # per-(rowtile, pair) gamma diags (bf16)

_BMAT = np.array(
    [
        [1 / 6, -3 / 6, 3 / 6, -1 / 6],
        [4 / 6, 0.0, -6 / 6, 3 / 6],
        [1 / 6, 3 / 6, 3 / 6, -3 / 6],
        [0.0, 0.0, 0.0, 1 / 6],
    ],
    dtype=np.float64,
)  # [tap k, power m]


def _spline_c123(alpha, b, group_idx):
    """Exact per-channel telescoped cubic increments (fp64).
    Returns oc[C], A[C], c123[C, NSEG, 3]."""
    g = group_idx.astype(np.int64)
    alpha_pc = alpha.astype(np.float64)[g]
    oc = 15.5 * (b.astype(np.float64) + 1.0) + 1.0
    S = np.arange(NSEG)
    taps = np.clip(S[:, None] - 2 + np.arange(4)[None, :], 0, K - 1)
    A4 = alpha_pc[:, taps]
    P = np.einsum("csk,km->csm", A4, _BMAT)
    return oc, P[:, 0, 0], P[..., 1:4]


def _F_eval(v, c123_c):
    out = np.zeros_like(v)
    for s in range(NSEG):
        r = np.clip(v - s, 0.0, 1.0)
        c1, c2, c3 = c123_c[s]
        out += ((c3 * r + c2) * r + c1) * r
    return out


def _bf16(x):
    import ml_dtypes
    return np.asarray(x, dtype=np.float32).astype(ml_dtypes.bfloat16).astype(np.float32)


_NPTS = 257
_RGRID = np.linspace(0.0, 1.0, _NPTS)
_MBASIS = np.stack([_RGRID, _RGRID**2, _RGRID**3, _RGRID**4], axis=1)


def _fit_channel(c123_c, oc_c, gamma_floor_frac=0.05):
    """Per-segment normalized quartic fit for one channel.
    Returns gammas[NSEGP] (bf16-exact fp32), a123[NSEGP,3] (fp32)."""
    gammas = np.zeros(NSEGP, np.float32)
    a123 = np.zeros((NSEGP, 3), np.float32)
    drift = 0.0
    F0 = 0.0
    for p, (S, w) in enumerate(zip(STARTS, WIDTHS)):
        v = S + w * _RGRID
        Fv = _F_eval(v, c123_c)
        target = Fv - F0 - drift
        z = (v - oc_c) / 15.5
        wgt = np.exp(-0.5 * z * z) + 1e-3
        wgt[-1] *= 1e4
        sw = np.sqrt(wgt)
        coef, *_ = np.linalg.lstsq(_MBASIS * sw[:, None], target * sw, rcond=None)
        c1, c2, c3, c4 = coef
        scale = max(np.max(np.abs(coef)), 1e-12)
        gmin = gamma_floor_frac * scale
        gam = c4 if abs(c4) >= gmin else (gmin if c4 >= 0 else -gmin)
        gam = float(_bf16(gam))
        resid = target - gam * _MBASIS[:, 3]
        coef3, *_ = np.linalg.lstsq(_MBASIS[:, :3] * sw[:, None], resid * sw, rcond=None)
        c1, c2, c3 = coef3
        a1 = np.float32(c1 / gam)
        a2 = np.float32(c2 / gam)
        a3 = np.float32(c3 / gam)
        gammas[p] = gam
        a123[p] = (a1, a2, a3)
        # device-exact saturated increment at r=1 (fp32 horner, bf16 out)
        m = np.float32(1.0)
        y = np.float32(((m + a3) * m + a2) * m + a1) * m
        Gdev = float(np.float32(np.float32(gam) * _bf16(y)))
        drift += Gdev - float(target[-1])
        F0 = Fv[-1]
    return gammas, a123


_TABLE_CACHE = {}


def build_tables(alpha, a, b, id_gain, bias, group_idx):
    """Returns (tab[ROWT,128,NTAB] f32, wts32[NW32,128,128] f32,
    wtsbf[NWBF,128,128] bf16-as-f32)."""
    key = (alpha.tobytes(), a.tobytes(), b.tobytes(), id_gain.tobytes(),
           bias.tobytes(), group_idx.tobytes())
    if key in _TABLE_CACHE:
        return _TABLE_CACHE[key]
    a64 = a.astype(np.float64)
    assert np.all(a64 == a64[0]), "fast path needs uniform a (ACT scale is imm)"
    assert abs(a64[0] - 1.0) < 1e-12, "tables assume a == 1 (fold into scale otherwise)"
    oc, A, c123 = _spline_c123(alpha, b, group_idx)
    bias2 = bias.astype(np.float64) + A

    gammas = np.zeros((C, NSEGP), np.float32)
    a123 = np.zeros((C, NSEGP, 3), np.float32)
    for c in range(C):
        gammas[c], a123[c] = _fit_channel(c123[c], oc[c])

    tab = np.zeros((ROWT, 128, NTAB), dtype=np.float32)
    wts32 = np.zeros((NW32, 128, 128), dtype=np.float32)
    wtsbf = np.zeros((NWBF, 128, 128), dtype=np.float32)
    for t in range(ROWT):
        ch = (t * 128 + np.arange(128)) % C
        for p, (S, w) in enumerate(zip(STARTS, WIDTHS)):
            tab[t, :, 4 * p + 0] = (oc[ch] - S) / w
            tab[t, :, 4 * p + 1] = a123[ch, p, 0]
            tab[t, :, 4 * p + 2] = a123[ch, p, 1]
            tab[t, :, 4 * p + 3] = a123[ch, p, 2]
            wtsbf[t * NSEGP + p] = np.diag(gammas[ch, p])
        wts32[2 * t] = np.diag(id_gain.astype(np.float64)[ch])
        wts32[2 * t + 1] = np.diag(bias2[ch])
    out = (tab, wts32, wtsbf)
    _TABLE_CACHE[key] = out
    return out


def host_emulate(x_rows, t, tab, wts32, wtsbf):
    """Numpy fp32 emulation of the device program for one row-tile."""
    f = np.float32
    ig = np.diag(wts32[2 * t])[:, None]
    b2 = np.diag(wts32[2 * t + 1])[:, None]
    acc = ig * x_rows + b2
    for p, (S, w) in enumerate(zip(STARTS, WIDTHS)):
        h = np.maximum(f(15.5 / w) * x_rows + tab[t, :, 4 * p + 0, None], f(0))
        m = np.minimum(h, f(1))
        a1 = tab[t, :, 4 * p + 1, None]
        a2 = tab[t, :, 4 * p + 2, None]
        a3 = tab[t, :, 4 * p + 3, None]
        y = _bf16((((m + a3) * m + a2) * m + a1) * m)
        gam = np.diag(wtsbf[t * NSEGP + p])[:, None]
        acc = acc + gam * y
    return acc


_PROG_CACHE = {}


def _get_custom_op():
    from concourse.dve_spec import (Spec, Src0, C0, C1, C3, One,
                                    minn, lower, _spill_c3_to_src1)
    from concourse import dve_ops
    from concourse.dve_ops import DveOp, OPS
    from concourse.dve_uop import DveOpSpec

    for op in OPS:
        if op.name == "KAN_Q4":
            return op

    r = minn(Src0, One)
    body = _spill_c3_to_src1((((r + C3) * r + C1) * r + C0) * r)

    def ref(in0, in1, s0, s1, imm2):
        m = np.minimum(in0, np.float32(1.0)).astype(np.float32)
        return ((((m + in1) * m + s1) * m + s0) * m).astype(np.float32)

    spec = Spec(body=body, reference=ref)
    shas = {}
    for ver in ("v3", "v4"):
        tmp = DveOpSpec(name="KAN_Q4", opcode=0, uops=lower(spec, ver=ver), rd1_en=True)
        shas[ver] = tmp.sha(ver)
    op = DveOp("KAN_Q4", spec, subdim=False, uops_sha=shas)
    row = dve_ops._CUSTOM_DVE_ROW_BASE + len(OPS)
    assert row < 0x20
    OPS.append(op)
    dve_ops.CUSTOM_DVE_SPECS[op.name] = spec
    dve_ops._SUB_OPCODE_FOR_NAME[op.name] = row
    assert dve_ops.get_dve_sub_opcode("KAN_Q4") == row
    return op


def _build_program():
    repeat = int(os.environ.get("KAN_REPEAT", "1"))
    key = ("prog", NCOL, repeat, os.environ.get("KAN_SKIP", ""))
    if key in _PROG_CACHE:
        return _PROG_CACHE[key]

    import concourse.bacc as bacc
    import concourse.mybir as mybir
    from concourse.tile import TileContext

    kan_op = _get_custom_op()

    nc = bacc.Bacc("TRN2", target_bir_lowering=False, debug=False, num_devices=NCORES)
    x_d = nc.dram_tensor("x", [ROWS, FREE], mybir.dt.float32, kind="ExternalInput").ap()
    tab_d = nc.dram_tensor("tab", [ROWT * 128, NTAB], mybir.dt.float32, kind="ExternalInput").ap()
    w32_d = nc.dram_tensor("wts32", [NW32 * 128, 128], mybir.dt.float32, kind="ExternalInput").ap()
    wbf_d = nc.dram_tensor("wtsbf", [NWBF * 128, 128], mybir.dt.bfloat16, kind="ExternalInput").ap()
    out_d = nc.dram_tensor("out", [ROWS, FREE], mybir.dt.float32, kind="ExternalOutput").ap()

    with TileContext(nc) as tc:
        with (
            tc.tile_pool(name="tabp", bufs=ROWT) as tabp,
            tc.tile_pool(name="w32p", bufs=NW32) as w32p,
            tc.tile_pool(name="wbfp", bufs=NWBF) as wbfp,
            tc.tile_pool(name="onesp", bufs=1) as onesp,
            tc.tile_pool(name="xp", bufs=int(os.environ.get("KAN_XBUFS", "2"))) as xp,
            tc.tile_pool(name="hp", bufs=int(os.environ.get("KAN_HBUFS", "3"))) as hp,
            tc.tile_pool(name="yp", bufs=int(os.environ.get("KAN_YBUFS", str(NSEGP + 2)))) as yp,
            tc.tile_pool(name="outp", bufs=2) as outp,
            tc.tile_pool(name="psp", bufs=2, space="PSUM") as psp,
        ):
            tabs, w32s, wbfs = [], [], []
            for t in range(ROWT):
                tt = tabp.tile([128, NTAB], mybir.dt.float32, tag="tab")
                nc.sync.dma_start(tt[:], tab_d[t * 128:(t + 1) * 128, :])
                tabs.append(tt)
            for i in range(NW32):
                wt_ = w32p.tile([128, 128], mybir.dt.float32, tag="w32")
                nc.sync.dma_start(wt_[:], w32_d[i * 128:(i + 1) * 128, :])
                w32s.append(wt_)
            for i in range(NWBF):
                wt_ = wbfp.tile([128, 128], mybir.dt.bfloat16, tag="wbf")
                nc.sync.dma_start(wt_[:], wbf_d[i * 128:(i + 1) * 128, :])
                wbfs.append(wt_)
            ones = onesp.tile([128, MMF], mybir.dt.float32, tag="ones")
            nc.vector.memset(ones[:], 1.0)

            import contextlib
            loop_ctx = tc.For_i(0, repeat, 1) if repeat > 1 else contextlib.nullcontext()
            with loop_ctx:
                _emit_body(nc, tc, tabs, w32s, wbfs, ones, x_d, out_d,
                           xp, hp, yp, outp, psp, kan_op)

    nc.compile()
    _PROG_CACHE[key] = nc
    return nc


def _emit_body(nc, tc, tabs, w32s, wbfs, ones, x_d, out_d, xp, hp, yp, outp, psp, kan_op):
    import concourse.mybir as mybir

    relu_f = mybir.ActivationFunctionType.Relu
    copy_f = mybir.ActivationFunctionType.Copy
    YDT = mybir.dt.bfloat16
    skip = os.environ.get("KAN_SKIP", "")      # diagnostic: "pe" | "dve" | "act"

    def evac(prev):
        # evacuate a finished tile's PSUM and store it. Emitted one tile LATE
        # (after the NEXT tile's Relu/DVE ops) so the ACT engine's strict FIFO
        # reaches this instruction only after PE has finished the bank --
        # otherwise the evac head-of-line-blocks the next tile's Relus and
        # serializes the whole pipeline.
        pps, prs, pcs = prev
        outt = outp.tile([128, NCOL], mybir.dt.float32, tag="out")
        nc.scalar.activation(outt[:], pps[:], copy_f, bias=0.0)
        nc.sync.dma_start(out_d[prs, pcs], outt[:])

    prev = None
    for t in range(ROWT):
        tt = tabs[t]
        igd, b2d = w32s[2 * t], w32s[2 * t + 1]
        gds = wbfs[t * NSEGP:(t + 1) * NSEGP]
        for j in range(COLT):
            rs, cs = slice(t * 128, (t + 1) * 128), slice(j * NCOL, (j + 1) * NCOL)
            xt = xp.tile([128, NCOL], mybir.dt.float32, tag="x")
            nc.sync.dma_start(xt[:], x_d[rs, cs])
            ys = []
            for p, (S, w) in enumerate(zip(STARTS, WIDTHS)):
                ht = hp.tile([128, NCOL], mybir.dt.float32, tag="h")
                if skip != "act":
                    nc.scalar.activation(
                        ht[:], xt[:], relu_f,
                        bias=tt[:, 4 * p:4 * p + 1], scale=float(15.5 / w),
                    )
                else:
                    nc.vector.memset(ht[:], 0.0)
                y = yp.tile([128, NCOL], YDT, tag="y")
                if skip != "dve":
                    nc.vector._custom_dve(
                        kan_op, out=y[:], in0=ht[:],
                        in1=tt[:, 4 * p + 3:4 * p + 4],
                        s0=tt[:, 4 * p + 1:4 * p + 2],
                        s1=tt[:, 4 * p + 2:4 * p + 3],
                        imm2=0.0,
                    )
                else:
                    nc.vector.memset(y[:], 0.0)
                ys.append(y)
            if prev is not None:
                evac(prev)
            ps = psp.tile([128, NCOL], mybir.dt.float32, tag="ps")
            if skip != "pe":
                # same-PSUM-bank bursts: all NSEGP+2 accumulating matmuls for one
                # 512-wide bank run back-to-back (avoids HAM bank-cycling throttle)
                for m in range(NMM):
                    ms = slice(m * MMF, (m + 1) * MMF)
                    nc.tensor.matmul(ps[:, ms], igd[:], xt[:, ms], start=True, stop=False)
                    nc.tensor.matmul(ps[:, ms], b2d[:], ones[:], start=False, stop=False)
                    for p in range(NSEGP):
                        nc.tensor.matmul(
                            ps[:, ms], gds[p][:], ys[p][:, ms],
                            start=False, stop=(p == NSEGP - 1),
                        )
            else:
                nc.vector.memset(ps[:], 0.0)
            prev = (ps, rs, cs)
    evac(prev)


def kernel(**inputs):
    x = np.asarray(inputs["x"], dtype=np.float32)
    tab, wts32, wtsbf = build_tables(
        np.asarray(inputs["alpha"]), np.asarray(inputs["a"]), np.asarray(inputs["b"]),
        np.asarray(inputs["id_gain"]), np.asarray(inputs["bias"]),
        np.asarray(inputs["group_idx"]),
    )
    from concourse import bass_utils
    import ml_dtypes

    nc = _build_program()
    tab_flat = np.ascontiguousarray(tab.reshape(ROWT * 128, NTAB))
    w32_flat = np.ascontiguousarray(wts32.reshape(NW32 * 128, 128))
    wbf_flat = np.ascontiguousarray(wtsbf.reshape(NWBF * 128, 128).astype(ml_dtypes.bfloat16))
    xs = x.reshape(NCORES, B // NCORES, C, H, W)
    in_maps = [
        {"x": np.ascontiguousarray(xs[i].reshape(ROWS, FREE)), "tab": tab_flat,
         "wts32": w32_flat, "wtsbf": wbf_flat}
        for i in range(NCORES)
    ]
    trace = bool(int(os.environ.get("KAN_TRACE", "0")))
    res = bass_utils.run_bass_kernel_spmd(
        nc, in_maps, list(range(NCORES)), trace=trace,
        tmpdir=os.environ.get("KAN_TMPDIR") or None,
    )
    if trace and res.exec_time_ns is not None:
        print(f"HW exec time: {res.exec_time_ns} ns")
    out = np.stack([res.results[i]["out"] for i in range(NCORES)])
    return np.ascontiguousarray(out.reshape(B, C, H, W).astype(np.float32))


# revision 19
# speedup vs baseline: 1.1886x; 1.1886x over previous
"""KAN group-spline kernel for Trainium2 (8 NeuronCores, data-parallel over batch).

Math: out = id_gain[c]*x + F_c(v) + bias[c], v = 15.5*a*x + oc_c, F_c = channel's
cubic spline (32-knot uniform B-spline, constant outside v in [0,33]).

v2 — wide-segment quartic telescoping (8 segments instead of 33):

  F_c(v) = A_c + sum_p gamma_{c,p} * q_{c,p}(r_p),   r_p = clamp((v-S_p)/w_p, 0, 1)
  q(m)   = m^4 + a3 m^3 + a2 m^2 + a1 m    (normalized: leading coeff == 1)

The quartic over w_p=4..5 knots is a host-side weighted LSQ fit of the exact
spline increment (rel err ~7e-3 vs 2e-2 budget); saturation is exact via
telescoping + host-side drift correction of the device's bf16-rounded
endpoint values.

Engine plan per 128xNCOL tile (segments independent -> full pipelining):
  ACT : h_p = Relu((15.5/w_p)*x + (oc_c - S_p)/w_p)   [per-partition bias AP]
  DVE : y_p = q(min(h_p, 1))      [custom 7-stage TTSS op "KAN_Q4";
        s0=a1, s1=a2, in1=a3 spill APs; bf16 out]
  PE  : psum  = diag(ig) @ x_chunk   [fp32, start]
        psum += diag(bias+A) @ ones  [fp32]
        psum += diag(gamma_p) @ y_p  [bf16, 8x]
  ACT : out_sbuf = Copy(psum)        [evacuate]

8 DVE passes + 9 ACT passes + 10 PE matmul-chunks per tile, vs the v1
33/2/35 split — DVE was the 1x-mode bottleneck (8-stage custom ops can't
use 2x/4x perf modes). Measured 1144136 ns (vs 3806968 ns baseline), rel
err 6.5e-3. NOTE: a 7-segment layout ([5,5,4,4,5,5,5], rel 1.0e-2) measured
SLOWER (1.38ms) despite fewer passes — do not "optimize" the pass count
down without re-measuring; folding bias2 into an Identity-evac with AP
bias also regressed (1.50ms).
"""

import os
import numpy as np

B, C, H, W = 16, 192, 128, 128
K, G = 32, 32
NSEG = 33
NCORES = 8
ROWS = (B // NCORES) * C           # 384 rows per core
FREE = H * W                       # 16384
NCOL = int(os.environ.get("KAN_NCOL", "2048"))
COLT = FREE // NCOL
ROWT = ROWS // 128                 # 3
MMF = 512                          # matmul free chunk / PSUM bank (fp32 out)
NMM = NCOL // MMF

WIDTHS = tuple(int(w) for w in os.environ.get("KAN_WIDTHS", "4,4,4,4,4,4,4,5").split(","))
assert sum(WIDTHS) == NSEG
NSEGP = len(WIDTHS)
STARTS = tuple(int(s) for s in np.concatenate([[0], np.cumsum(WIDTHS)])[:-1])

# tab layout per pair p: [actbias, a1, a2, a3]
NTAB = 4 * NSEGP
NW32 = 2 * ROWT                    # per-rowtile {ig, bias2} diags (fp32)
NWBF = ROWT * NSEGP                # per-(rowtile, pair) gamma diags (bf16)

_BMAT = np.array(
    [
        [1 / 6, -3 / 6, 3 / 6, -1 / 6],
        [4 / 6, 0.0, -6 / 6, 3 / 6],
        [1 / 6, 3 / 6, 3 / 6, -3 / 6],
        [0.0, 0.0, 0.0, 1 / 6],
    ],
    dtype=np.float64,
)  # [tap k, power m]


def _spline_c123(alpha, b, group_idx):
    """Exact per-channel telescoped cubic increments (fp64).
    Returns oc[C], A[C], c123[C, NSEG, 3]."""
    g = group_idx.astype(np.int64)
    alpha_pc = alpha.astype(np.float64)[g]
    oc = 15.5 * (b.astype(np.float64) + 1.0) + 1.0
    S = np.arange(NSEG)
    taps = np.clip(S[:, None] - 2 + np.arange(4)[None, :], 0, K - 1)
    A4 = alpha_pc[:, taps]
    P = np.einsum("csk,km->csm", A4, _BMAT)
    return oc, P[:, 0, 0], P[..., 1:4]


def _F_eval(v, c123_c):
    out = np.zeros_like(v)
    for s in range(NSEG):
        r = np.clip(v - s, 0.0, 1.0)
        c1, c2, c3 = c123_c[s]
        out += ((c3 * r + c2) * r + c1) * r
    return out


def _bf16(x):
    import ml_dtypes
    return np.asarray(x, dtype=np.float32).astype(ml_dtypes.bfloat16).astype(np.float32)


_NPTS = 257
_RGRID = np.linspace(0.0, 1.0, _NPTS)
_MBASIS = np.stack([_RGRID, _RGRID**2, _RGRID**3, _RGRID**4], axis=1)


def _fit_channel(c123_c, oc_c, gamma_floor_frac=0.05):
    """Per-segment normalized quartic fit for one channel.
    Returns gammas[NSEGP] (bf16-exact fp32), a123[NSEGP,3] (fp32)."""
    gammas = np.zeros(NSEGP, np.float32)
    a123 = np.zeros((NSEGP, 3), np.float32)
    drift = 0.0
    F0 = 0.0
    for p, (S, w) in enumerate(zip(STARTS, WIDTHS)):
        v = S + w * _RGRID
        Fv = _F_eval(v, c123_c)
        target = Fv - F0 - drift
        z = (v - oc_c) / 15.5
        wgt = np.exp(-0.5 * z * z) + 1e-3
        wgt[-1] *= 1e4
        sw = np.sqrt(wgt)
        coef, *_ = np.linalg.lstsq(_MBASIS * sw[:, None], target * sw, rcond=None)
        c1, c2, c3, c4 = coef
        scale = max(np.max(np.abs(coef)), 1e-12)
        gmin = gamma_floor_frac * scale
        gam = c4 if abs(c4) >= gmin else (gmin if c4 >= 0 else -gmin)
        gam = float(_bf16(gam))
        resid = target - gam * _MBASIS[:, 3]
        coef3, *_ = np.linalg.lstsq(_MBASIS[:, :3] * sw[:, None], resid * sw, rcond=None)
        c1, c2, c3 = coef3
        a1 = np.float32(c1 / gam)
        a2 = np.float32(c2 / gam)
        a3 = np.float32(c3 / gam)
        gammas[p] = gam
        a123[p] = (a1, a2, a3)
        # device-exact saturated increment at r=1 (fp32 horner, bf16 out)
        m = np.float32(1.0)
        y = np.float32(((m + a3) * m + a2) * m + a1) * m
        Gdev = float(np.float32(np.float32(gam) * _bf16(y)))
        drift += Gdev - float(target[-1])
        F0 = Fv[-1]
    return gammas, a123


_TABLE_CACHE = {}


def build_tables(alpha, a, b, id_gain, bias, group_idx):
    """Returns (tab[ROWT,128,NTAB] f32, wts32[NW32,128,128] f32,
    wtsbf[NWBF,128,128] bf16-as-f32)."""
    key = (alpha.tobytes(), a.tobytes(), b.tobytes(), id_gain.tobytes(),
           bias.tobytes(), group_idx.tobytes())
    if key in _TABLE_CACHE:
        return _TABLE_CACHE[key]
    a64 = a.astype(np.float64)
    assert np.all(a64 == a64[0]), "fast path needs uniform a (ACT scale is imm)"
    assert abs(a64[0] - 1.0) < 1e-12, "tables assume a == 1 (fold into scale otherwise)"
    oc, A, c123 = _spline_c123(alpha, b, group_idx)
    bias2 = bias.astype(np.float64) + A

    gammas = np.zeros((C, NSEGP), np.float32)
    a123 = np.zeros((C, NSEGP, 3), np.float32)
    for c in range(C):
        gammas[c], a123[c] = _fit_channel(c123[c], oc[c])

    tab = np.zeros((ROWT, 128, NTAB), dtype=np.float32)
    wts32 = np.zeros((NW32, 128, 128), dtype=np.float32)
    wtsbf = np.zeros((NWBF, 128, 128), dtype=np.float32)
    for t in range(ROWT):
        ch = (t * 128 + np.arange(128)) % C
        for p, (S, w) in enumerate(zip(STARTS, WIDTHS)):
            tab[t, :, 4 * p + 0] = (oc[ch] - S) / w
            tab[t, :, 4 * p + 1] = a123[ch, p, 0]
            tab[t, :, 4 * p + 2] = a123[ch, p, 1]
            tab[t, :, 4 * p + 3] = a123[ch, p, 2]
            wtsbf[t * NSEGP + p] = np.diag(gammas[ch, p])
        wts32[2 * t] = np.diag(id_gain.astype(np.float64)[ch])
        wts32[2 * t + 1] = np.diag(bias2[ch])
    out = (tab, wts32, wtsbf)
    _TABLE_CACHE[key] = out
    return out


def host_emulate(x_rows, t, tab, wts32, wtsbf):
    """Numpy fp32 emulation of the device program for one row-tile."""
    f = np.float32
    ig = np.diag(wts32[2 * t])[:, None]
    b2 = np.diag(wts32[2 * t + 1])[:, None]
    acc = ig * x_rows + b2
    for p, (S, w) in enumerate(zip(STARTS, WIDTHS)):
        h = np.maximum(f(15.5 / w) * x_rows + tab[t, :, 4 * p + 0, None], f(0))
        m = np.minimum(h, f(1))
        a1 = tab[t, :, 4 * p + 1, None]
        a2 = tab[t, :, 4 * p + 2, None]
        a3 = tab[t, :, 4 * p + 3, None]
        y = _bf16((((m + a3) * m + a2) * m + a1) * m)
        gam = np.diag(wtsbf[t * NSEGP + p])[:, None]
        acc = acc + gam * y
    return acc


_PROG_CACHE = {}


def _get_custom_op():
    from concourse.dve_spec import (Spec, Src0, C0, C1, C3, One,
                                    minn, lower, _spill_c3_to_src1)
    from concourse import dve_ops
    from concourse.dve_ops import DveOp, OPS
    from concourse.dve_uop import DveOpSpec

    for op in OPS:
        if op.name == "KAN_Q4":
            return op

    r = minn(Src0, One)
    body = _spill_c3_to_src1((((r + C3) * r + C1) * r + C0) * r)

    def ref(in0, in1, s0, s1, imm2):
        m = np.minimum(in0, np.float32(1.0)).astype(np.float32)
        return ((((m + in1) * m + s1) * m + s0) * m).astype(np.float32)

    spec = Spec(body=body, reference=ref)
    shas = {}
    for ver in ("v3", "v4"):
        tmp = DveOpSpec(name="KAN_Q4", opcode=0, uops=lower(spec, ver=ver), rd1_en=True)
        shas[ver] = tmp.sha(ver)
    op = DveOp("KAN_Q4", spec, subdim=False, uops_sha=shas)
    row = dve_ops._CUSTOM_DVE_ROW_BASE + len(OPS)
    assert row < 0x20
    OPS.append(op)
    dve_ops.CUSTOM_DVE_SPECS[op.name] = spec
    dve_ops._SUB_OPCODE_FOR_NAME[op.name] = row
    assert dve_ops.get_dve_sub_opcode("KAN_Q4") == row
    return op


def _build_program():
    repeat = int(os.environ.get("KAN_REPEAT", "1"))
    key = ("prog", NCOL, repeat, os.environ.get("KAN_SKIP", ""))
    if key in _PROG_CACHE:
        return _PROG_CACHE[key]

    import concourse.bacc as bacc
    import concourse.mybir as mybir
    from concourse.tile import TileContext

    kan_op = _get_custom_op()

    nc = bacc.Bacc("TRN2", target_bir_lowering=False, debug=False, num_devices=NCORES)
    x_d = nc.dram_tensor("x", [ROWS, FREE], mybir.dt.float32, kind="ExternalInput").ap()
    tab_d = nc.dram_tensor("tab", [ROWT * 128, NTAB], mybir.dt.float32, kind="ExternalInput").ap()
    w32_d = nc.dram_tensor("wts32", [NW32 * 128, 128], mybir.dt.float32, kind="ExternalInput").ap()
    wbf_d = nc.dram_tensor("wtsbf", [NWBF * 128, 128], mybir.dt.bfloat16, kind="ExternalInput").ap()
    out_d = nc.dram_tensor("out", [ROWS, FREE], mybir.dt.float32, kind="ExternalOutput").ap()

    with TileContext(nc) as tc:
        with (
            tc.tile_pool(name="tabp", bufs=ROWT) as tabp,
            tc.tile_pool(name="w32p", bufs=NW32) as w32p,
            tc.tile_pool(name="wbfp", bufs=NWBF) as wbfp,
            tc.tile_pool(name="onesp", bufs=1) as onesp,
            tc.tile_pool(name="xp", bufs=int(os.environ.get("KAN_XBUFS", "2"))) as xp,
            tc.tile_pool(name="hp", bufs=int(os.environ.get("KAN_HBUFS", "3"))) as hp,
            tc.tile_pool(name="yp", bufs=int(os.environ.get("KAN_YBUFS", str(NSEGP + 2)))) as yp,
            tc.tile_pool(name="outp", bufs=2) as outp,
            tc.tile_pool(name="psp", bufs=2, space="PSUM") as psp,
        ):
            tabs, w32s, wbfs = [], [], []
            for t in range(ROWT):
                tt = tabp.tile([128, NTAB], mybir.dt.float32, tag="tab")
                nc.sync.dma_start(tt[:], tab_d[t * 128:(t + 1) * 128, :])
                tabs.append(tt)
            for i in range(NW32):
                wt_ = w32p.tile([128, 128], mybir.dt.float32, tag="w32")
                nc.sync.dma_start(wt_[:], w32_d[i * 128:(i + 1) * 128, :])
                w32s.append(wt_)
            for i in range(NWBF):
                wt_ = wbfp.tile([128, 128], mybir.dt.bfloat16, tag="wbf")
                nc.sync.dma_start(wt_[:], wbf_d[i * 128:(i + 1) * 128, :])
                wbfs.append(wt_)
            ones = onesp.tile([128, MMF], mybir.dt.float32, tag="ones")
            nc.vector.memset(ones[:], 1.0)

            import contextlib
            loop_ctx = tc.For_i(0, repeat, 1) if repeat > 1 else contextlib.nullcontext()
            with loop_ctx:
                _emit_body(nc, tc, tabs, w32s, wbfs, ones, x_d, out_d,
                           xp, hp, yp, outp, psp, kan_op)

    nc.compile()
    _PROG_CACHE[key] = nc
    return nc


def _emit_body(nc, tc, tabs, w32s, wbfs, ones, x_d, out_d, xp, hp, yp, outp, psp, kan_op):
    import concourse.mybir as mybir

    relu_f = mybir.ActivationFunctionType.Relu
    copy_f = mybir.ActivationFunctionType.Copy
    YDT = mybir.dt.bfloat16
    skip = os.environ.get("KAN_SKIP", "")      # diagnostic: "pe" | "dve" | "act"
    for t in range(ROWT):
        tt = tabs[t]
        igd, b2d = w32s[2 * t], w32s[2 * t + 1]
        gds = wbfs[t * NSEGP:(t + 1) * NSEGP]
        for j in range(COLT):
            rs, cs = slice(t * 128, (t + 1) * 128), slice(j * NCOL, (j + 1) * NCOL)
            xt = xp.tile([128, NCOL], mybir.dt.float32, tag="x")
            nc.sync.dma_start(xt[:], x_d[rs, cs])
            ys = []
            for p, (S, w) in enumerate(zip(STARTS, WIDTHS)):
                ht = hp.tile([128, NCOL], mybir.dt.float32, tag="h")
                if skip != "act":
                    nc.scalar.activation(
                        ht[:], xt[:], relu_f,
                        bias=tt[:, 4 * p:4 * p + 1], scale=float(15.5 / w),
                    )
                else:
                    nc.vector.memset(ht[:], 0.0)
                y = yp.tile([128, NCOL], YDT, tag="y")
                if skip != "dve":
                    nc.vector._custom_dve(
                        kan_op, out=y[:], in0=ht[:],
                        in1=tt[:, 4 * p + 3:4 * p + 4],
                        s0=tt[:, 4 * p + 1:4 * p + 2],
                        s1=tt[:, 4 * p + 2:4 * p + 3],
                        imm2=0.0,
                    )
                else:
                    nc.vector.memset(y[:], 0.0)
                ys.append(y)
            ps = psp.tile([128, NCOL], mybir.dt.float32, tag="ps")
            if skip != "pe":
                # same-PSUM-bank bursts: all NSEGP+2 accumulating matmuls for one
                # 512-wide bank run back-to-back (avoids HAM bank-cycling throttle)
                for m in range(NMM):
                    ms = slice(m * MMF, (m + 1) * MMF)
                    nc.tensor.matmul(ps[:, ms], igd[:], xt[:, ms], start=True, stop=False)
                    nc.tensor.matmul(ps[:, ms], b2d[:], ones[:], start=False, stop=False)
                    for p in range(NSEGP):
                        nc.tensor.matmul(
                            ps[:, ms], gds[p][:], ys[p][:, ms],
                            start=False, stop=(p == NSEGP - 1),
                        )
            else:
                nc.vector.memset(ps[:], 0.0)
            outt = outp.tile([128, NCOL], mybir.dt.float32, tag="out")
            nc.scalar.activation(outt[:], ps[:], copy_f, bias=0.0)
            nc.sync.dma_start(out_d[rs, cs], outt[:])


def kernel(**inputs):
    x = np.asarray(inputs["x"], dtype=np.float32)
    tab, wts32, wtsbf = build_tables(
        np.asarray(inputs["alpha"]), np.asarray(inputs["a"]), np.asarray(inputs["b"]),
        np.asarray(inputs["id_gain"]), np.asarray(inputs["bias"]),
        np.asarray(inputs["group_idx"]),
    )
    from concourse import bass_utils
    import ml_dtypes

    nc = _build_program()
    tab_flat = np.ascontiguousarray(tab.reshape(ROWT * 128, NTAB))
    w32_flat = np.ascontiguousarray(wts32.reshape(NW32 * 128, 128))
    wbf_flat = np.ascontiguousarray(wtsbf.reshape(NWBF * 128, 128).astype(ml_dtypes.bfloat16))
    xs = x.reshape(NCORES, B // NCORES, C, H, W)
    in_maps = [
        {"x": np.ascontiguousarray(xs[i].reshape(ROWS, FREE)), "tab": tab_flat,
         "wts32": w32_flat, "wtsbf": wbf_flat}
        for i in range(NCORES)
    ]
    trace = bool(int(os.environ.get("KAN_TRACE", "0")))
    res = bass_utils.run_bass_kernel_spmd(
        nc, in_maps, list(range(NCORES)), trace=trace,
        tmpdir=os.environ.get("KAN_TMPDIR") or None,
    )
    if trace and res.exec_time_ns is not None:
        print(f"HW exec time: {res.exec_time_ns} ns")
    out = np.stack([res.results[i]["out"] for i in range(NCORES)])
    return np.ascontiguousarray(out.reshape(B, C, H, W).astype(np.float32))
